# revision 1
# baseline (speedup 1.0000x reference)
"""Trainium2 Bass kernel for nn_DecoupledAttentionWeight.

Computes the five projections q_sem/k_sem/q_geo/k_geo/v of x, applies RoPE to
the geo paths, the per-head sigmoid gate + per-path scaling to q (folded into
the projection weights host-side), and returns (q_cat, k_cat, vh) shaped
(B, H, T, 128) each.

Sharding over 8 NeuronCores: 2-way data-parallel over batch (batches {0,1} /
{2,3}) x 4-way tensor-parallel over heads (4 heads per core). Each core runs
one big [8192 x 2048] @ [2048 x 1536] matmul in fp32r (20-bit float: full PE
speed, ~1e-4 rel err) with the per-head output columns packed as
[q_sem|q_geo|k_sem|k_geo|v] so the sem||geo concat is free, then RoPE on the
geo strips via DVE with broadcast access patterns.
"""
import math
import os
import sys

import numpy as np

for _p in ("/opt/trn_rl_repo", os.path.expanduser("~/.axon_site/_ro/trn_rl_repo")):
    if os.path.isdir(_p) and _p not in sys.path:
        sys.path.insert(0, _p)

import concourse.bacc as bacc
import concourse.mybir as mybir
import concourse.tile as tile
from concourse.bass_utils import run_bass_kernel_spmd

# Problem config (hardcoded from the nn.Module init)
D_MODEL = 2048
N_HEADS = 16
SEM_HD = 64
GEO_HD = 64
HEAD_DIM = 128
ROPE_DIM = 64
ROPE_HALF = ROPE_DIM // 2  # 32
ROPE_BASE = 10000.0
B, T = 4, 4096

# Sharding: 2 row groups (2 batches each) x 4 head groups (4 heads each)
N_CORES = 8
RG, HG = 2, 4
ROWS_PER_CORE = (B * T) // RG          # 8192
HEADS_PER_CORE = N_HEADS // HG         # 4
BLK = SEM_HD + GEO_HD + SEM_HD + GEO_HD + HEAD_DIM  # 384 cols per head
N_CORE = HEADS_PER_CORE * BLK          # 1536
K_TILES = D_MODEL // 128               # 16
M_TILES = ROWS_PER_CORE // 128         # 64
SLAB_MT = 2                            # m_tiles per input DMA slab
SLAB_ROWS = SLAB_MT * 128              # 256
N_SLABS = M_TILES // SLAB_MT           # 32
CHUNK = 512                            # psum bank / matmul moving size
N_CHUNKS = N_CORE // CHUNK             # 3
COS_SLOTS = T // 128                   # 32 distinct cos/sin row-tiles

_f32 = mybir.dt.float32
_f32r = mybir.dt.float32r


def _round_fp32r(a: np.ndarray) -> np.ndarray:
    """Round-to-nearest-even fp32 -> fp32r (11-bit mantissa in top 20 bits)."""
    u = np.ascontiguousarray(a, dtype=np.float32).view(np.uint32)
    lsb = (u >> 12) & np.uint32(1)
    r = (u + np.uint32(0x7FF) + lsb) & np.uint32(0xFFFFF000)
    return r.view(np.float32)


def _build_nc():
    nc = bacc.Bacc("TRN2", target_bir_lowering=False, debug=False, num_devices=1)
    xt_d = nc.dram_tensor("xt", [D_MODEL, ROWS_PER_CORE], _f32r, kind="ExternalInput")
    w_d = nc.dram_tensor("w", [D_MODEL, N_CORE], _f32r, kind="ExternalInput")
    cos_d = nc.dram_tensor("cos", [T, ROPE_HALF], _f32, kind="ExternalInput")
    sin_d = nc.dram_tensor("sin", [T, ROPE_HALF], _f32, kind="ExternalInput")
    q_d = nc.dram_tensor(
        "q", [HEADS_PER_CORE, ROWS_PER_CORE, HEAD_DIM], _f32, kind="ExternalOutput"
    )
    k_d = nc.dram_tensor(
        "k", [HEADS_PER_CORE, ROWS_PER_CORE, HEAD_DIM], _f32, kind="ExternalOutput"
    )
    v_d = nc.dram_tensor(
        "v", [HEADS_PER_CORE, ROWS_PER_CORE, HEAD_DIM], _f32, kind="ExternalOutput"
    )

    with tile.TileContext(nc) as tc:
        with (
            tc.tile_pool(name="wp", bufs=1) as wp,
            tc.tile_pool(name="xp", bufs=3) as xp,
            tc.tile_pool(name="trig", bufs=1) as trigp,
            tc.tile_pool(name="stg", bufs=3) as stgp,
            tc.tile_pool(name="tmp", bufs=2) as tmpp,
            tc.tile_pool(name="ps", bufs=2, space="PSUM") as ps,
        ):
            xt_kd = xt_d.ap().rearrange("(k p) m -> p k m", p=128)
            slab_tiles = {}

            def load_slab(s):
                if s not in slab_tiles:
                    t = xp.tile([128, K_TILES * SLAB_ROWS], _f32r, tag="xt")
                    # scalar HWDGE ring: parallel to the W/output DMAs on sync
                    nc.scalar.dma_start(
                        t[:].rearrange("p (k m) -> p k m", k=K_TILES),
                        xt_kd[:, :, s * SLAB_ROWS:(s + 1) * SLAB_ROWS],
                    )
                    slab_tiles[s] = t
                return slab_tiles[s]

            # First x slab must land before any MM can run: issue it ahead
            # of the W loads on the scalar ring.
            load_slab(0)

            # Weights resident, one tile per k so the first m_tile's k-loop
            # can start as soon as W[k=0] lands instead of stalling on the
            # whole 12.6MB load.
            w_kd = w_d.ap().rearrange("(k p) n -> k p n", p=128)
            w_tiles = []
            for k in range(K_TILES):
                wt = wp.tile([128, N_CORE], _f32r, tag=f"w{k}")
                nc.sync.dma_start(wt[:], w_kd[k])
                w_tiles.append(wt)

            # cos/sin tables resident: [128, slot(32) x 32]
            cos_sb = trigp.tile([128, COS_SLOTS * ROPE_HALF], _f32, tag="cos")
            nc.sync.dma_start(
                cos_sb[:].rearrange("p (s c) -> p s c", s=COS_SLOTS),
                cos_d.ap().rearrange("(s p) c -> p s c", p=128),
            )
            sin_sb = trigp.tile([128, COS_SLOTS * ROPE_HALF], _f32, tag="sin")
            nc.sync.dma_start(
                sin_sb[:].rearrange("p (s c) -> p s c", s=COS_SLOTS),
                sin_d.ap().rearrange("(s p) c -> p s c", p=128),
            )
            cos_v = cos_sb[:].rearrange("p (s c) -> p s c", s=COS_SLOTS)
            sin_v = sin_sb[:].rearrange("p (s c) -> p s c", s=COS_SLOTS)

            for s in range(N_SLABS):
                xt_sb = load_slab(s)
                xt_v = xt_sb[:].rearrange("p (k m) -> p k m", k=K_TILES)

                for i in range(SLAB_MT):
                    mt = s * SLAB_MT + i
                    psum = ps.tile([128, N_CORE], _f32, name="psum", tag="psum")
                    # chunk-outer / k-inner: 16 consecutive accumulating MMs
                    # into the same PSUM bank (no per-MM bank switching)
                    for c in range(N_CHUNKS):
                        for k in range(K_TILES):
                            nc.tensor.matmul(
                                psum[:, c * CHUNK:(c + 1) * CHUNK],
                                xt_v[:, k, i * 128:(i + 1) * 128],
                                w_tiles[k][:, c * CHUNK:(c + 1) * CHUNK],
                                start=(k == 0),
                                stop=(k == K_TILES - 1),
                            )

                    # Postprocess: RoPE on geo strips, copy the rest.
                    # Per-head col layout: [qsem 64|qgeo 64|ksem 64|kgeo 64|v 128]
                    # viewed as (h, t3, c): t3=0 -> q(128), 1 -> k(128), 2 -> v(128)
                    pv = psum[:, :].rearrange(
                        "p (h t c) -> p h t c", h=HEADS_PER_CORE, t=3
                    )
                    stg = stgp.tile([128, N_CORE], _f32, tag="stg")
                    sv = stg[:].rearrange(
                        "p (h t c) -> p h t c", h=HEADS_PER_CORE, t=3
                    )
                    slot = mt % COS_SLOTS
                    cos_bc = (
                        cos_v[:, slot, :]
                        .unsqueeze(1)
                        .unsqueeze(1)
                        .broadcast_to([128, HEADS_PER_CORE, 2, ROPE_HALF])
                    )
                    sin_bc = (
                        sin_v[:, slot, :]
                        .unsqueeze(1)
                        .unsqueeze(1)
                        .broadcast_to([128, HEADS_PER_CORE, 2, ROPE_HALF])
                    )
                    x1 = pv[:, :, 0:2, 64:96]
                    x2 = pv[:, :, 0:2, 96:128]
                    shp = [128, HEADS_PER_CORE, 2, ROPE_HALF]
                    t1 = tmpp.tile(shp, _f32, tag="t1")
                    t2 = tmpp.tile(shp, _f32, tag="t2")
                    t3 = tmpp.tile(shp, _f32, tag="t3")
                    t4 = tmpp.tile(shp, _f32, tag="t4")
                    nc.vector.tensor_mul(t1[:], x1, cos_bc)
                    nc.vector.tensor_mul(t2[:], x2, sin_bc)
                    nc.vector.tensor_mul(t3[:], x2, cos_bc)
                    nc.vector.tensor_mul(t4[:], x1, sin_bc)
                    nc.vector.tensor_sub(sv[:, :, 0:2, 64:96], t1[:], t2[:])
                    nc.vector.tensor_add(sv[:, :, 0:2, 96:128], t3[:], t4[:])
                    # sem halves of q and k
                    nc.any.tensor_copy(sv[:, :, 0:2, 0:64], pv[:, :, 0:2, 0:64])
                    # v
                    nc.any.tensor_copy(sv[:, :, 2, :], pv[:, :, 2, :])

                    m0 = mt * 128
                    for t3_idx, out_d in ((0, q_d), (1, k_d), (2, v_d)):
                        nc.sync.dma_start(
                            out_d.ap()[:, m0:m0 + 128, :].transpose([1, 0, 2]),
                            sv[:, :, t3_idx, :],
                        )

    nc.compile()
    return nc


_NC_CACHE = None
LAST_RESULTS = None


def _get_nc():
    global _NC_CACHE
    if _NC_CACHE is None:
        _NC_CACHE = _build_nc()
    return _NC_CACHE


def _host_tables(pos_offset):
    """cos/sin tables computed exactly as the reference does (f32 jax ops)."""
    import jax
    import jax.numpy as jnp

    with jax.default_device(jax.devices("cpu")[0]):
        inv_freq = ROPE_BASE ** (
            -jnp.arange(0, ROPE_HALF, dtype=jnp.float32) * (2.0 / ROPE_DIM)
        )
        pos = jnp.arange(T, dtype=jnp.float32) + jnp.float32(pos_offset)
        ang = pos[:, None] * inv_freq[None, :]
        cos = np.asarray(jnp.cos(ang), dtype=np.float32)
        sin = np.asarray(jnp.sin(ang), dtype=np.float32)
    return np.ascontiguousarray(cos), np.ascontiguousarray(sin)


def _gate(gate_logit):
    import jax
    import jax.numpy as jnp

    with jax.default_device(jax.devices("cpu")[0]):
        g = np.asarray(
            jax.nn.sigmoid(jnp.asarray(gate_logit, dtype=jnp.float32)),
            dtype=np.float32,
        )
    return g


def kernel(x, wq_sem, wk_sem, wq_geo, wk_geo, wv, gate_logit, pos_offset):
    x = np.asarray(x, dtype=np.float32)
    wq_sem = np.asarray(wq_sem, dtype=np.float32)
    wk_sem = np.asarray(wk_sem, dtype=np.float32)
    wq_geo = np.asarray(wq_geo, dtype=np.float32)
    wk_geo = np.asarray(wk_geo, dtype=np.float32)
    wv = np.asarray(wv, dtype=np.float32)
    pos_off = int(np.asarray(pos_offset))

    g = _gate(gate_logit)  # (16,)
    sem_scale = np.float32(1.0 / math.sqrt(float(SEM_HD)))
    geo_scale = np.float32(1.0 / math.sqrt(float(GEO_HD)))
    q_sem_col = (np.float32(2.0) * g * sem_scale).astype(np.float32)   # per head
    q_geo_col = ((np.float32(2.0) - np.float32(2.0) * g) * geo_scale).astype(
        np.float32
    )

    # Per-core weight slabs, cols per head: [qsem|qgeo|ksem|kgeo|v]
    w_cores = []
    for hg in range(HG):
        cols = []
        for hl in range(HEADS_PER_CORE):
            h = hg * HEADS_PER_CORE + hl
            cols.append(wq_sem[:, h * 64:(h + 1) * 64] * q_sem_col[h])
            cols.append(wq_geo[:, h * 64:(h + 1) * 64] * q_geo_col[h])
            cols.append(wk_sem[:, h * 64:(h + 1) * 64])
            cols.append(wk_geo[:, h * 64:(h + 1) * 64])
            cols.append(wv[:, h * 128:(h + 1) * 128])
        w_cores.append(_round_fp32r(np.concatenate(cols, axis=1)))

    # x^T, rounded to fp32r, split into the two row groups
    xr = _round_fp32r(x.reshape(B * T, D_MODEL))
    xt = xr.T  # (2048, 16384) view
    xt_rg = [
        np.ascontiguousarray(xt[:, rg * ROWS_PER_CORE:(rg + 1) * ROWS_PER_CORE])
        for rg in range(RG)
    ]

    cos, sin = _host_tables(pos_off)

    in_maps = []
    for core in range(N_CORES):
        rg, hg = core // HG, core % HG
        in_maps.append(
            {"xt": xt_rg[rg], "w": w_cores[hg], "cos": cos, "sin": sin}
        )

    nc = _get_nc()
    res = run_bass_kernel_spmd(nc, in_maps, list(range(N_CORES)))
    global LAST_RESULTS
    LAST_RESULTS = res

    q_cat = np.empty((B, N_HEADS, T, HEAD_DIM), np.float32)
    k_cat = np.empty((B, N_HEADS, T, HEAD_DIM), np.float32)
    vh = np.empty((B, N_HEADS, T, HEAD_DIM), np.float32)
    for core in range(N_CORES):
        rg, hg = core // HG, core % HG
        r = res.results[core]
        for name, dst in (("q", q_cat), ("k", k_cat), ("v", vh)):
            # (4, 8192, 128) -> (heads, b_local, T, 128)
            a = r[name].reshape(HEADS_PER_CORE, 2, T, HEAD_DIM)
            dst[
                rg * 2:(rg + 1) * 2,
                hg * HEADS_PER_CORE:(hg + 1) * HEADS_PER_CORE,
            ] = a.transpose(1, 0, 2, 3)
    return q_cat, k_cat, vh



# revision 2
# speedup vs baseline: 1.1014x; 1.1014x over previous
"""Trainium2 Bass kernel for nn_DecoupledAttentionWeight.

Computes the five projections q_sem/k_sem/q_geo/k_geo/v of x, applies RoPE to
the geo paths, the per-head sigmoid gate + per-path scaling to q (folded into
the projection weights host-side), and returns (q_cat, k_cat, vh) shaped
(B, H, T, 128) each.

Sharding over 8 NeuronCores: 2-way data-parallel over batch (batches {0,1} /
{2,3}) x 4-way tensor-parallel over heads (4 heads per core). Each core runs
one big [8192 x 2048] @ [2048 x 1536] matmul in bf16 (full PE speed, ~2e-3
rel err against the f32 reference) with the per-head output columns packed as
[q_sem|q_geo|k_sem|k_geo|v] so the sem||geo concat is free, then RoPE on the
geo strips via DVE with broadcast access patterns.

v2 changes vs the fp32r baseline (764 us):
 - bf16 x/w/out halve all DMA traffic (startup head was DMA-bound).
 - Host-side layouts give per-partition-contiguous DMA descriptors
   (x slabs 8 KiB, w 3 KiB, cos/sin 4 KiB, out 3 KiB runs) -- the old
   1 KiB/512 B descriptors were packet-rate-bound (~100 GB/s/queue).
 - k-outer/chunk-inner matmul order: the first m-tile consumes w[k]
   incrementally as the weight tiles land instead of stalling on the
   full weight load; stationary x-tile is reused across the 3 chunks.
 - Weight tiles split across both HWDGE rings (even k on sync, odd on
   scalar) so the weight load finishes in ~half the time.
 - One fused output DMA per m-tile ([128 x 1536] bf16) instead of three
   strided f32 writes.
"""
import math
import os
import sys

import numpy as np

for _p in ("/opt/trn_rl_repo", os.path.expanduser("~/.axon_site/_ro/trn_rl_repo")):
    if os.path.isdir(_p) and _p not in sys.path:
        sys.path.insert(0, _p)

import ml_dtypes

import concourse.bacc as bacc
import concourse.mybir as mybir
import concourse.tile as tile
from concourse.bass_utils import run_bass_kernel_spmd

# Problem config (hardcoded from the nn.Module init)
D_MODEL = 2048
N_HEADS = 16
SEM_HD = 64
GEO_HD = 64
HEAD_DIM = 128
ROPE_DIM = 64
ROPE_HALF = ROPE_DIM // 2  # 32
ROPE_BASE = 10000.0
B, T = 4, 4096

# Sharding: 2 row groups (2 batches each) x 4 head groups (4 heads each)
N_CORES = 8
RG, HG = 2, 4
ROWS_PER_CORE = (B * T) // RG          # 8192
HEADS_PER_CORE = N_HEADS // HG         # 4
BLK = SEM_HD + GEO_HD + SEM_HD + GEO_HD + HEAD_DIM  # 384 cols per head
N_CORE = HEADS_PER_CORE * BLK          # 1536
K_TILES = D_MODEL // 128               # 16
M_TILES = ROWS_PER_CORE // 128         # 64
SLAB_MT = 2                            # m_tiles per input DMA slab
SLAB_ROWS = SLAB_MT * 128              # 256
N_SLABS = M_TILES // SLAB_MT           # 32
SLAB_W = K_TILES * SLAB_ROWS           # 4096 bf16 elems per partition
CHUNK = 512                            # psum bank / matmul moving size
N_CHUNKS = N_CORE // CHUNK             # 3
COS_SLOTS = T // 128                   # 32 distinct cos/sin row-tiles

_f32 = mybir.dt.float32
_bf16 = mybir.dt.bfloat16
_bf = ml_dtypes.bfloat16


def _build_nc():
    nc = bacc.Bacc("TRN2", target_bir_lowering=False, debug=False, num_devices=1)
    xs_d = nc.dram_tensor("xs", [128, N_SLABS, SLAB_W], _bf16, kind="ExternalInput")
    w_d = nc.dram_tensor("w", [K_TILES, 128, N_CORE], _bf16, kind="ExternalInput")
    cos_d = nc.dram_tensor("cos", [128, COS_SLOTS * ROPE_HALF], _f32, kind="ExternalInput")
    sin_d = nc.dram_tensor("sin", [128, COS_SLOTS * ROPE_HALF], _f32, kind="ExternalInput")
    out_d = nc.dram_tensor(
        "out", [ROWS_PER_CORE, N_CORE], _bf16, kind="ExternalOutput"
    )

    with tile.TileContext(nc) as tc:
        with (
            tc.tile_pool(name="wp", bufs=1) as wp,
            tc.tile_pool(name="xp", bufs=3) as xp,
            tc.tile_pool(name="trig", bufs=1) as trigp,
            tc.tile_pool(name="stg", bufs=3) as stgp,
            tc.tile_pool(name="tmp", bufs=2) as tmpp,
            tc.tile_pool(name="ps", bufs=2, space="PSUM") as ps,
        ):
            # cos/sin tables resident: [128, slot(32) x 32], contiguous rows
            cos_sb = trigp.tile([128, COS_SLOTS * ROPE_HALF], _f32, tag="cos")
            nc.scalar.dma_start(cos_sb[:], cos_d.ap())
            sin_sb = trigp.tile([128, COS_SLOTS * ROPE_HALF], _f32, tag="sin")
            nc.scalar.dma_start(sin_sb[:], sin_d.ap())
            cos_v = cos_sb[:].rearrange("p (s c) -> p s c", s=COS_SLOTS)
            sin_v = sin_sb[:].rearrange("p (s c) -> p s c", s=COS_SLOTS)

            slab_tiles = {}

            def load_slab(s):
                if s not in slab_tiles:
                    t = xp.tile([128, SLAB_W], _bf16, tag="xt")
                    nc.scalar.dma_start(t[:], xs_d.ap()[:, s, :])
                    slab_tiles[s] = t
                return slab_tiles[s]

            # First x slab ahead of the odd-k weight loads on the scalar ring.
            load_slab(0)

            # Weights resident, one tile per k, split across both HWDGE rings
            # (even k on sync, odd k on scalar) so the k-sequential consumer
            # in the first m-tile is fed from two queues in parallel.
            w_tiles = [None] * K_TILES
            for k in range(0, K_TILES, 2):
                wt = wp.tile([128, N_CORE], _bf16, tag=f"w{k}")
                nc.sync.dma_start(wt[:], w_d.ap()[k])
                w_tiles[k] = wt
            for k in range(1, K_TILES, 2):
                wt = wp.tile([128, N_CORE], _bf16, tag=f"w{k}")
                nc.scalar.dma_start(wt[:], w_d.ap()[k])
                w_tiles[k] = wt

            for s in range(N_SLABS):
                xt_sb = load_slab(s)
                if s + 1 < N_SLABS:
                    load_slab(s + 1)
                if s + 2 < N_SLABS:
                    load_slab(s + 2)
                xt_v = xt_sb[:].rearrange("p (k m) -> p k m", k=K_TILES)

                for i in range(SLAB_MT):
                    mt = s * SLAB_MT + i
                    psum = ps.tile([128, N_CORE], _f32, name="psum", tag="psum")
                    # k-outer / chunk-inner: stationary x-tile reused across
                    # the 3 chunks; the first m-tile consumes w[k] in arrival
                    # order during the weight load.
                    for k in range(K_TILES):
                        for c in range(N_CHUNKS):
                            nc.tensor.matmul(
                                psum[:, c * CHUNK:(c + 1) * CHUNK],
                                xt_v[:, k, i * 128:(i + 1) * 128],
                                w_tiles[k][:, c * CHUNK:(c + 1) * CHUNK],
                                start=(k == 0),
                                stop=(k == K_TILES - 1),
                            )

                    # Postprocess: RoPE on geo strips, copy the rest.
                    # Per-head col layout: [qsem 64|qgeo 64|ksem 64|kgeo 64|v 128]
                    # viewed as (h, t3, c): t3=0 -> q(128), 1 -> k(128), 2 -> v(128)
                    pv = psum[:, :].rearrange(
                        "p (h t c) -> p h t c", h=HEADS_PER_CORE, t=3
                    )
                    stg = stgp.tile([128, N_CORE], _bf16, tag="stg")
                    sv = stg[:].rearrange(
                        "p (h t c) -> p h t c", h=HEADS_PER_CORE, t=3
                    )
                    slot = mt % COS_SLOTS
                    cos_bc = (
                        cos_v[:, slot, :]
                        .unsqueeze(1)
                        .unsqueeze(1)
                        .broadcast_to([128, HEADS_PER_CORE, 2, ROPE_HALF])
                    )
                    sin_bc = (
                        sin_v[:, slot, :]
                        .unsqueeze(1)
                        .unsqueeze(1)
                        .broadcast_to([128, HEADS_PER_CORE, 2, ROPE_HALF])
                    )
                    x1 = pv[:, :, 0:2, 64:96]
                    x2 = pv[:, :, 0:2, 96:128]
                    shp = [128, HEADS_PER_CORE, 2, ROPE_HALF]
                    t1 = tmpp.tile(shp, _f32, tag="t1")
                    t2 = tmpp.tile(shp, _f32, tag="t2")
                    t3 = tmpp.tile(shp, _f32, tag="t3")
                    t4 = tmpp.tile(shp, _f32, tag="t4")
                    nc.vector.tensor_mul(t1[:], x1, cos_bc)
                    nc.vector.tensor_mul(t2[:], x2, sin_bc)
                    nc.vector.tensor_mul(t3[:], x2, cos_bc)
                    nc.vector.tensor_mul(t4[:], x1, sin_bc)
                    nc.vector.tensor_sub(sv[:, :, 0:2, 64:96], t1[:], t2[:])
                    nc.vector.tensor_add(sv[:, :, 0:2, 96:128], t3[:], t4[:])
                    # sem halves of q and k
                    nc.any.tensor_copy(sv[:, :, 0:2, 0:64], pv[:, :, 0:2, 0:64])
                    # v
                    nc.any.tensor_copy(sv[:, :, 2, :], pv[:, :, 2, :])

                    m0 = mt * 128
                    nc.sync.dma_start(out_d.ap()[m0:m0 + 128, :], stg[:])

    nc.compile()
    return nc


_NC_CACHE = None
LAST_RESULTS = None


def _get_nc():
    global _NC_CACHE
    if _NC_CACHE is None:
        _NC_CACHE = _build_nc()
    return _NC_CACHE


def _host_tables(pos_offset):
    """cos/sin tables computed exactly as the reference does (f32 jax ops)."""
    import jax
    import jax.numpy as jnp

    with jax.default_device(jax.devices("cpu")[0]):
        inv_freq = ROPE_BASE ** (
            -jnp.arange(0, ROPE_HALF, dtype=jnp.float32) * (2.0 / ROPE_DIM)
        )
        pos = jnp.arange(T, dtype=jnp.float32) + jnp.float32(pos_offset)
        ang = pos[:, None] * inv_freq[None, :]
        cos = np.asarray(jnp.cos(ang), dtype=np.float32)
        sin = np.asarray(jnp.sin(ang), dtype=np.float32)
    # [T, 32] -> [p, slot*32 + c], row t = slot*128 + p
    cos = np.ascontiguousarray(
        cos.reshape(COS_SLOTS, 128, ROPE_HALF).transpose(1, 0, 2).reshape(128, -1)
    )
    sin = np.ascontiguousarray(
        sin.reshape(COS_SLOTS, 128, ROPE_HALF).transpose(1, 0, 2).reshape(128, -1)
    )
    return cos, sin


def _gate(gate_logit):
    import jax

    g = np.asarray(
        jax.nn.sigmoid(np.asarray(gate_logit, dtype=np.float32)), dtype=np.float32
    )
    return g


def kernel(x, wq_sem, wk_sem, wq_geo, wk_geo, wv, gate_logit, pos_offset):
    x = np.asarray(x, dtype=np.float32)
    wq_sem = np.asarray(wq_sem, dtype=np.float32)
    wk_sem = np.asarray(wk_sem, dtype=np.float32)
    wq_geo = np.asarray(wq_geo, dtype=np.float32)
    wk_geo = np.asarray(wk_geo, dtype=np.float32)
    wv = np.asarray(wv, dtype=np.float32)
    pos_off = int(np.asarray(pos_offset))

    g = _gate(gate_logit)  # (16,)
    sem_scale = np.float32(1.0 / math.sqrt(float(SEM_HD)))
    geo_scale = np.float32(1.0 / math.sqrt(float(GEO_HD)))
    q_sem_col = (np.float32(2.0) * g * sem_scale).astype(np.float32)   # per head
    q_geo_col = ((np.float32(2.0) - np.float32(2.0) * g) * geo_scale).astype(
        np.float32
    )

    # Per-core weight slabs, cols per head: [qsem|qgeo|ksem|kgeo|v],
    # laid out [k, p, n] so each k-tile DMA reads 3 KiB/partition runs.
    w_cores = []
    for hg in range(HG):
        cols = []
        for hl in range(HEADS_PER_CORE):
            h = hg * HEADS_PER_CORE + hl
            cols.append(wq_sem[:, h * 64:(h + 1) * 64] * q_sem_col[h])
            cols.append(wq_geo[:, h * 64:(h + 1) * 64] * q_geo_col[h])
            cols.append(wk_sem[:, h * 64:(h + 1) * 64])
            cols.append(wk_geo[:, h * 64:(h + 1) * 64])
            cols.append(wv[:, h * 128:(h + 1) * 128])
        wc = np.concatenate(cols, axis=1).astype(_bf)       # (2048, 1536)
        w_cores.append(np.ascontiguousarray(wc.reshape(K_TILES, 128, N_CORE)))

    # x -> per-row-group slab layout [p, s, k*256+m] (8 KiB contiguous
    # per partition per slab)
    xb = x.reshape(RG, N_SLABS, SLAB_ROWS, K_TILES, 128).astype(_bf)
    xs_rg = [
        np.ascontiguousarray(xb[rg].transpose(3, 0, 2, 1).reshape(128, N_SLABS, SLAB_W))
        for rg in range(RG)
    ]

    cos, sin = _host_tables(pos_off)

    in_maps = []
    for core in range(N_CORES):
        rg, hg = core // HG, core % HG
        in_maps.append(
            {"xs": xs_rg[rg], "w": w_cores[hg], "cos": cos, "sin": sin}
        )

    nc = _get_nc()
    res = run_bass_kernel_spmd(nc, in_maps, list(range(N_CORES)))
    global LAST_RESULTS
    LAST_RESULTS = res

    q_cat = np.empty((B, N_HEADS, T, HEAD_DIM), np.float32)
    k_cat = np.empty((B, N_HEADS, T, HEAD_DIM), np.float32)
    vh = np.empty((B, N_HEADS, T, HEAD_DIM), np.float32)
    for core in range(N_CORES):
        rg, hg = core // HG, core % HG
        # (8192, 1536) bf16 -> (b_local, T, h, t3, c)
        a = np.asarray(res.results[core]["out"]).astype(np.float32)
        a = a.reshape(2, T, HEADS_PER_CORE, 3, HEAD_DIM)
        for t3_idx, dst in ((0, q_cat), (1, k_cat), (2, vh)):
            dst[
                rg * 2:(rg + 1) * 2,
                hg * HEADS_PER_CORE:(hg + 1) * HEADS_PER_CORE,
            ] = a[:, :, :, t3_idx, :].transpose(0, 2, 1, 3)
    return q_cat, k_cat, vh
